# revision 23
# baseline (speedup 1.0000x reference)
"""GaussianImage splat kernel for 8 trn2 NeuronCores (v2).

Math: for gaussian n and pixel p,
  S[n,p] = -0.5 * q[n,p] + ln(norm[n])  (q = mahalanobis quadratic form)
is a degree-2 polynomial in pixel coords: S = coef[6,n]^T @ basis[6,p]
(basis rows u^2, v^2, uv, u, v, 1 with u,v = coords - 0.5).
prob = exp(S); img[p,c] = sum_n prob[n,p]*w[n,c]; out = sigmoid(img/max prob).

fp32 matmul on trn2 PE runs LOW_HIGH dual-pass (~10x slower than bf16), so
the exponent contraction uses an exact 3-piece bf16 split: c = c0+c1+c2,
b = b0+b1+b2 (bf16 pieces), keeping the 6 products with i+j<=2 gives
~2^-27 relative error — better than an fp32 matmul. The 6 (ci,bj) pairs
are stacked on the contraction axis: one K=48 bf16 matmul.

Sharding: pixels split across 8 cores (each core handles all 512 gaussians
on its 32768 pixels): no image all-reduce; only a 512B AllReduce(max).

Per core, per (pixel-group g of 1024 px, gaussian-chunk j of 128):
  PE:  S_psum[128, 1024] = coefP_j[48,128]^T @ basisP[48, 1024]  (bf16)
  ACT: prob[128, 1024] = exp(S_psum) -> SBUF bf16
  DVE: running max: run = max(run, prob)   (tensor_tensor, 2x bf16)
  PE:  img[3(q), 512] += w_j[128,3]^T @ prob[:, t*512:...]  (bf16)
       with q = g%4 selecting PSUM col-group 32q via tile_position, so 4
       groups accumulate into one [128, 1024] psum tile (2 banks).
Per super (4 groups): one DVE copy [128,1024] psum->sbuf acc.
Tail: reduce run -> [128,1], AllReduce(max), reciprocal, broadcast,
  sigmoid(acc * 1/pmax) in one [128, 8192] ACT op, DMA 4x[3,8192] out.
"""

import sys

if "/opt/trn_rl_repo" not in sys.path:
    sys.path.insert(0, "/opt/trn_rl_repo")

import numpy as np
import ml_dtypes

N_GAUSS = 512
H = W = 512
HW = H * W
NCORES = 8
PX = HW // NCORES          # 32768 pixels per core
F = 1536                   # pixels per exp group (3 psum banks); the 3
                           # 512-px blocks of a group land in PE col-groups
                           # 0/32/64 of one [128,512] img psum tile (col
                           # group 96 is a broken HW quadrant)
FB = 512                   # pixels per block / img matmul
NG = (PX + F - 1) // F     # 22 groups (21x1536 + 1x512)
NCHUNK = 4                 # gaussian chunks of 128
KP = 128                   # 6 bf16 piece-pairs x 8 rows, zero-padded to 128:
                           # partial-row-group matmuls (K=48) measure ~486ns
                           # vs ~283ns for full-array K=128 on HW
ACC_C = NG * FB            # acc columns (one [128,512] img tile per group)

_cache = {}


def _build_nc():
    import concourse.mybir as mybir
    from concourse import bacc, tile

    f32 = mybir.dt.float32
    bf16 = mybir.dt.bfloat16
    AF = mybir.ActivationFunctionType
    ALU = mybir.AluOpType

    nc = bacc.Bacc("TRN2", num_devices=NCORES)

    basis_d = nc.dram_tensor("basis", [KP, PX], bf16, kind="ExternalInput")
    coef_d = nc.dram_tensor("coef", [KP, 512], bf16, kind="ExternalInput")
    w_d = nc.dram_tensor("wrgb", [128, 3 * NCHUNK], bf16, kind="ExternalInput")
    out_d = nc.dram_tensor("img", [9, ACC_C], f32, kind="ExternalOutput")
    if _cache.get("debug"):
        dbg_img_d = nc.dram_tensor("dbg_img", [128, ACC_C], f32, kind="ExternalOutput")

    with tile.TileContext(nc) as tc:
        with (
            tc.tile_pool(name="const", bufs=1) as constp,
            tc.tile_pool(name="probp", bufs=8) as probp,
            tc.tile_pool(name="psS", bufs=2, space="PSUM") as psS,
            tc.tile_pool(name="psI", bufs=2, space="PSUM") as psI,
        ):
            basis_s = constp.tile([KP, PX], bf16, tag="basis")
            coef_s = constp.tile([KP, 512], bf16, tag="coef")
            w_s = constp.tile([128, 3 * NCHUNK], bf16, tag="w")
            acc_s = constp.tile([128, ACC_C], f32, tag="acc")
            zrow = constp.tile([1, 128], bf16, tag="zrow")

            # stage inputs: weights first (first matmul needs them), then
            # basis in graduated chunks so group 0 lands fast while later
            # groups amortize the ~0.7us per-DMA issue cost on the SP queue.
            # coef split by gaussian chunk: the first matmul only needs
            # chunk 0's columns, so it can start ~4us earlier
            for j4 in range(4):
                cs = slice(128 * j4, 128 * j4 + 128)
                nc.sync.dma_start(out=coef_s[:, cs], in_=coef_d[:, cs])
            nc.sync.dma_start(out=w_s[:], in_=w_d[:])
            for p4 in range(4):
                ps = slice(32 * p4, 32 * p4 + 32)
                nc.sync.dma_start(out=basis_s[ps, 0:512], in_=basis_d[ps, 0:512])
            edges = [512]
            for sz in [512, 1024, 1024, 1024, 2048, 2048]:
                edges.append(edges[-1] + sz)
            while edges[-1] < PX:
                edges.append(min(PX, edges[-1] + 4096))
            for e0, e1 in zip(edges[:-1], edges[1:]):
                nc.sync.dma_start(
                    out=basis_s[:, e0:e1], in_=basis_d[:, e0:e1]
                )
            nc.vector.memset(zrow[:], 0.0)

            out2 = constp.tile([128, ACC_C], f32, tag="out2")

            def emit_img(p):
                # deferred a few (g,j) iterations: prob is long since ready,
                # so PE never stalls waiting on ACT (software pipeline).
                pimg, pj, pprob, pf, pflush, pg = p
                for t in range(pf // FB):
                    nc.tensor.matmul(
                        pimg[32 * t:32 * t + 3, :],
                        w_s[:, 3 * pj:3 * pj + 3],
                        pprob[:, t * FB:(t + 1) * FB],
                        start=False,
                        stop=pflush and not _cache.get("debug"),
                        skip_group_check=True,
                    )
                if pflush:
                    nc.vector.tensor_copy(
                        acc_s[:, pg * FB:(pg + 1) * FB], pimg[:]
                    )
                    if pg % 2 == 1 or pg == NG - 1:
                        m0 = pg - 1 if pg % 2 == 1 else pg
                        cs = slice(m0 * FB, (pg + 1) * FB)
                        nc.scalar.activation(out2[:, cs], acc_s[:, cs], AF.Sigmoid)
                        for t in range(3):
                            nc.sync.dma_start(
                                out=out_d[3 * t:3 * t + 3, cs],
                                in_=out2[32 * t:32 * t + 3, cs],
                            )

            from collections import deque

            dq = deque()
            k = 0
            for g in range(NG):
                p0 = g * F
                f = min(F, PX - p0)
                img = psI.tile([128, FB], f32, tag="img")
                # zeroing matmul: writes 0 to every element of the bank and
                # sets has_written on all 128 partitions, so the col-group
                # accumulation below is pure accumulate regardless of the
                # HW's bank-clear partition scope.
                nc.tensor.matmul(
                    img[:, :],
                    zrow[:],
                    basis_s[0:1, 0:FB],
                    start=True,
                    stop=False,
                    skip_group_check=True,
                )
                for j in range(NCHUNK):
                    S = psS.tile([128, f], f32, tag="S")
                    prob = probp.tile([128, f], bf16, tag="prob")
                    for t in range(f // 512):
                        nc.tensor.matmul(
                            S[:, t * 512:(t + 1) * 512],
                            coef_s[:, j * 128:(j + 1) * 128],
                            basis_s[:, p0 + t * 512: p0 + (t + 1) * 512],
                            start=True,
                            stop=True,
                        )
                    nc.scalar.activation(prob[:], S[:], AF.Exp)
                    dq.append((img, j, prob, f, j == NCHUNK - 1, g))
                    # emit deferred img matmuls two iterations at a time so
                    # PE sees batched same-shape runs (measured 222ns/MM
                    # batched vs 282 alternating vs 380 dependent)
                    if k % 2 == 1 and len(dq) >= 4:
                        emit_img(dq.popleft())
                        emit_img(dq.popleft())
                    k += 1
            while dq:
                emit_img(dq.popleft())

            # 1/pmax is folded into the coefficients on the host (exact grid
            # max of a concave quadratic, computed per scanline), so no
            # device-side max and no collective at all.
            if _cache.get("debug"):
                nc.sync.dma_start(out=dbg_img_d[:], in_=acc_s[:])


    nc.compile()
    return nc


def _split3(x):
    """Exact 3-piece bf16 decomposition of an fp32 array."""
    bf = ml_dtypes.bfloat16
    x = x.astype(np.float32)
    p0 = x.astype(bf).astype(np.float32)
    r = x - p0
    p1 = r.astype(bf).astype(np.float32)
    p2 = r - p1
    return p0.astype(bf), p1.astype(bf), p2.astype(bf)


# piece index per pair for coef (i) and basis (j): products with i+j<=2
_PAIR_I = [0, 0, 1, 0, 1, 2]
_PAIR_J = [0, 1, 0, 2, 1, 0]


def _host_prep(mean, alpha, scale, theta, rgb, pixels):
    """Per-gaussian polynomial coefficients + per-pixel basis, bf16-split."""
    mean = np.asarray(mean, np.float64)
    alpha = np.asarray(alpha, np.float64)
    scale = np.asarray(scale, np.float64)
    theta = np.asarray(theta, np.float64)
    rgb = np.asarray(rgb, np.float64)
    pixels = np.asarray(pixels, np.float64)

    ta = 2.0 * np.pi * theta[:, 0]
    c, s = np.cos(ta), np.sin(ta)
    sx2 = scale[:, 0] ** 2
    sy2 = scale[:, 1] ** 2
    a = c * c * sx2 + s * s * sy2
    b = c * s * (sx2 - sy2)
    d = s * s * sx2 + c * c * sy2
    det = a * d - b * b
    A = d / det
    B = a / det
    C = -2.0 * b / det
    lognorm = -np.log(2.0 * np.pi) - 0.5 * np.log(det)
    mux = mean[:, 0, 0] - 0.5
    muy = mean[:, 1, 0] - 0.5

    coef = np.zeros((8, N_GAUSS), np.float64)
    coef[0] = -0.5 * A
    coef[1] = -0.5 * B
    coef[2] = -0.5 * C
    coef[3] = A * mux + 0.5 * C * muy
    coef[4] = B * muy + 0.5 * C * mux
    coef[5] = -0.5 * (A * mux**2 + B * muy**2 + C * mux * muy) + lognorm

    # Exact max of S over the pixel grid, folded into the constant row so
    # the device computes normalized splats directly (no device max, no
    # collective). Exactness: per scanline v=v_h, S is a concave parabola
    # in u, whose grid argmax is the grid point nearest its vertex.
    ug = np.unique(pixels[0, :, 0]) - 0.5 if (
        np.all(pixels[:, :, 0] == pixels[0:1, :, 0])
        and np.all(pixels[:, :, 1] == pixels[:, 0:1, 1])
    ) else None
    if ug is not None and ug.size > 1:
        vg = pixels[:, 0, 1] - 0.5                    # (H,)
        du = ug[1] - ug[0]
        r0 = coef[0][:, None]                         # (N,1)
        bu = coef[2][:, None] * vg[None, :] + coef[3][:, None]   # (N,H)
        cu = (coef[1][:, None] * vg[None, :] ** 2
              + coef[4][:, None] * vg[None, :] + coef[5][:, None])
        ustar = np.clip(-bu / (2.0 * r0), ug[0], ug[-1])
        ustar = ug[0] + np.round((ustar - ug[0]) / du) * du
        smax = float((r0 * ustar**2 + bu * ustar + cu).max())
    else:
        # non-separable pixel grid: brute-force the max on the host
        u = pixels[:, :, 0].reshape(-1) - 0.5
        v = pixels[:, :, 1].reshape(-1) - 0.5
        bas = np.stack([u * u, v * v, u * v, u, v, np.ones_like(u)], 0)
        smax = -np.inf
        for i in range(0, HW, 65536):
            smax = max(smax, float((coef[:6].T @ bas[:, i:i + 65536]).max()))
    coef[5] -= smax
    coef = coef.astype(np.float32)

    u = pixels[:, :, 0].reshape(-1) - 0.5
    v = pixels[:, :, 1].reshape(-1) - 0.5
    basis = np.zeros((8, HW), np.float32)
    basis[0] = u * u
    basis[1] = v * v
    basis[2] = u * v
    basis[3] = u
    basis[4] = v
    basis[5] = 1.0

    cp = _split3(coef)      # 3 x (8, 512) bf16
    bp = _split3(basis)     # 3 x (8, HW) bf16
    coefP = np.zeros((KP, N_GAUSS), ml_dtypes.bfloat16)   # rows 48+ stay 0
    basisP = np.zeros((KP, HW), ml_dtypes.bfloat16)
    for p in range(6):
        coefP[8 * p:8 * p + 8] = cp[_PAIR_I[p]]
        basisP[8 * p:8 * p + 8] = bp[_PAIR_J[p]]

    wfull = (rgb * alpha).astype(np.float32)          # (512, 3)
    w_dev = np.ascontiguousarray(
        wfull.reshape(NCHUNK, 128, 3).transpose(1, 0, 2).reshape(128, 3 * NCHUNK)
    ).astype(ml_dtypes.bfloat16)
    return coefP, basisP, w_dev


def _assemble(results):
    """results[c]["img"] (9, NG*FB) -> full (H, W, 3) image."""
    out = np.empty((HW, 3), np.float32)
    for c in range(NCORES):
        arr = np.asarray(results[c]["img"], np.float32).reshape(3, 3, NG, FB)
        # arr[t, ch, g, u] -> pixel g*F + t*FB + u of this core
        core = arr.transpose(2, 0, 3, 1).reshape(NG * 3 * FB, 3)
        out[c * PX:(c + 1) * PX] = core[:PX]
    return out.reshape(H, W, 3)


def run(inputs, trace=False):
    from concourse.bass_utils import run_bass_kernel_spmd

    coefP, basisP, w_dev = _host_prep(**inputs)
    if "nc" not in _cache:
        _cache["nc"] = _build_nc()
    nc = _cache["nc"]

    in_maps = [
        {
            "basis": np.ascontiguousarray(basisP[:, c * PX:(c + 1) * PX]),
            "coef": coefP,
            "wrgb": w_dev,
        }
        for c in range(NCORES)
    ]
    res = run_bass_kernel_spmd(
        nc, in_maps, core_ids=list(range(NCORES)), trace=trace
    )
    return _assemble(res.results), res


def kernel(mean, alpha, scale, theta, rgb, pixels):
    out, _ = run(
        dict(mean=mean, alpha=alpha, scale=scale, theta=theta, rgb=rgb,
             pixels=pixels)
    )
    return out


# revision 24
# speedup vs baseline: 1.2136x; 1.2136x over previous
"""GaussianImage splat kernel for 8 trn2 NeuronCores (v2).

Math: for gaussian n and pixel p,
  S[n,p] = -0.5 * q[n,p] + ln(norm[n])  (q = mahalanobis quadratic form)
is a degree-2 polynomial in pixel coords: S = coef[6,n]^T @ basis[6,p]
(basis rows u^2, v^2, uv, u, v, 1 with u,v = coords - 0.5).
prob = exp(S); img[p,c] = sum_n prob[n,p]*w[n,c]; out = sigmoid(img/max prob).

fp32 matmul on trn2 PE runs LOW_HIGH dual-pass (~10x slower than bf16), so
the exponent contraction uses an exact 3-piece bf16 split: c = c0+c1+c2,
b = b0+b1+b2 (bf16 pieces), keeping the 6 products with i+j<=2 gives
~2^-27 relative error — better than an fp32 matmul. The 6 (ci,bj) pairs
are stacked on the contraction axis: one K=48 bf16 matmul.

Sharding: pixels split across 8 cores (each core handles all 512 gaussians
on its 32768 pixels): no image all-reduce; only a 512B AllReduce(max).

Per core, per (pixel-group g of 1024 px, gaussian-chunk j of 128):
  PE:  S_psum[128, 1024] = coefP_j[48,128]^T @ basisP[48, 1024]  (bf16)
  ACT: prob[128, 1024] = exp(S_psum) -> SBUF bf16
  DVE: running max: run = max(run, prob)   (tensor_tensor, 2x bf16)
  PE:  img[3(q), 512] += w_j[128,3]^T @ prob[:, t*512:...]  (bf16)
       with q = g%4 selecting PSUM col-group 32q via tile_position, so 4
       groups accumulate into one [128, 1024] psum tile (2 banks).
Per super (4 groups): one DVE copy [128,1024] psum->sbuf acc.
Tail: reduce run -> [128,1], AllReduce(max), reciprocal, broadcast,
  sigmoid(acc * 1/pmax) in one [128, 8192] ACT op, DMA 4x[3,8192] out.
"""

import sys

if "/opt/trn_rl_repo" not in sys.path:
    sys.path.insert(0, "/opt/trn_rl_repo")

import numpy as np
import ml_dtypes

N_GAUSS = 512
H = W = 512
HW = H * W
NCORES = 8
PX = HW // NCORES          # 32768 pixels per core
F = 1536                   # pixels per exp group (3 psum banks); the 3
                           # 512-px blocks of a group land in PE col-groups
                           # 0/32/64 of one [128,512] img psum tile (col
                           # group 96 is a broken HW quadrant)
FB = 512                   # pixels per block / img matmul
NG = (PX + F - 1) // F     # 22 groups (21x1536 + 1x512)
NCHUNK = 4                 # gaussian chunks of 128
KP = 128                   # 6 bf16 piece-pairs x 8 rows, zero-padded to 128:
                           # partial-row-group matmuls (K=48) measure ~486ns
                           # vs ~283ns for full-array K=128 on HW
ACC_C = NG * FB            # acc columns (one [128,512] img tile per group)

_cache = {}


def _build_nc():
    import concourse.mybir as mybir
    from concourse import bacc, tile

    f32 = mybir.dt.float32
    bf16 = mybir.dt.bfloat16
    AF = mybir.ActivationFunctionType
    ALU = mybir.AluOpType

    nc = bacc.Bacc("TRN2", num_devices=NCORES)

    basis_d = nc.dram_tensor("basis", [KP, PX], bf16, kind="ExternalInput")
    coef_d = nc.dram_tensor("coef", [KP, 512], bf16, kind="ExternalInput")
    w_d = nc.dram_tensor("wrgb", [128, 3 * NCHUNK], bf16, kind="ExternalInput")
    out_d = nc.dram_tensor("img", [9, ACC_C], f32, kind="ExternalOutput")
    if _cache.get("debug"):
        dbg_img_d = nc.dram_tensor("dbg_img", [128, ACC_C], f32, kind="ExternalOutput")

    with tile.TileContext(nc) as tc:
        with (
            tc.tile_pool(name="const", bufs=1) as constp,
            tc.tile_pool(name="probp", bufs=8) as probp,
            tc.tile_pool(name="psS", bufs=2, space="PSUM") as psS,
            tc.tile_pool(name="psI", bufs=2, space="PSUM") as psI,
        ):
            basis_s = constp.tile([KP, PX], bf16, tag="basis")
            coef_s = constp.tile([KP, 512], bf16, tag="coef")
            w_s = constp.tile([128, 3 * NCHUNK], bf16, tag="w")
            acc_s = constp.tile([128, ACC_C], f32, tag="acc")
            zrow = constp.tile([1, 128], bf16, tag="zrow")

            # stage inputs: weights first (first matmul needs them), then
            # basis in graduated chunks so group 0 lands fast while later
            # groups amortize the ~0.7us per-DMA issue cost on the SP queue.
            # coef split by gaussian chunk: the first matmul only needs
            # chunk 0's columns, so it can start ~4us earlier
            for j4 in range(4):
                cs = slice(128 * j4, 128 * j4 + 128)
                nc.sync.dma_start(out=coef_s[:, cs], in_=coef_d[:, cs])
            nc.sync.dma_start(out=w_s[:], in_=w_d[:])
            for p4 in range(4):
                ps = slice(32 * p4, 32 * p4 + 32)
                nc.sync.dma_start(out=basis_s[ps, 0:512], in_=basis_d[ps, 0:512])
            edges = [512]
            for sz in [512, 1024, 1024, 1024, 2048, 2048]:
                edges.append(edges[-1] + sz)
            while edges[-1] < PX:
                edges.append(min(PX, edges[-1] + 4096))
            for e0, e1 in zip(edges[:-1], edges[1:]):
                nc.sync.dma_start(
                    out=basis_s[:, e0:e1], in_=basis_d[:, e0:e1]
                )
            nc.vector.memset(zrow[:], 0.0)

            def emit_img(p):
                # deferred a few (g,j) iterations: prob is long since ready,
                # so PE never stalls waiting on ACT (software pipeline).
                pimg, pj, pprob, pf, pflush, pg = p
                for t in range(pf // FB):
                    nc.tensor.matmul(
                        pimg[32 * t:32 * t + 3, :],
                        w_s[:, 3 * pj:3 * pj + 3],
                        pprob[:, t * FB:(t + 1) * FB],
                        start=False,
                        stop=pflush and not _cache.get("debug"),
                        skip_group_check=True,
                    )
                if pflush:
                    nc.vector.tensor_copy(
                        acc_s[:, pg * FB:(pg + 1) * FB], pimg[:]
                    )

            from collections import deque

            dq = deque()
            k = 0
            for g in range(NG):
                p0 = g * F
                f = min(F, PX - p0)
                img = psI.tile([128, FB], f32, tag="img")
                # zeroing matmul: writes 0 to every element of the bank and
                # sets has_written on all 128 partitions, so the col-group
                # accumulation below is pure accumulate regardless of the
                # HW's bank-clear partition scope.
                nc.tensor.matmul(
                    img[:, :],
                    zrow[:],
                    basis_s[0:1, 0:FB],
                    start=True,
                    stop=False,
                    skip_group_check=True,
                )
                for j in range(NCHUNK):
                    S = psS.tile([128, f], f32, tag="S")
                    prob = probp.tile([128, f], bf16, tag="prob")
                    for t in range(f // 512):
                        nc.tensor.matmul(
                            S[:, t * 512:(t + 1) * 512],
                            coef_s[:, j * 128:(j + 1) * 128],
                            basis_s[:, p0 + t * 512: p0 + (t + 1) * 512],
                            start=True,
                            stop=True,
                        )
                    nc.scalar.activation(prob[:], S[:], AF.Exp)
                    dq.append((img, j, prob, f, j == NCHUNK - 1, g))
                    # emit deferred img matmuls two iterations at a time so
                    # PE sees batched same-shape runs (measured 222ns/MM
                    # batched vs 282 alternating vs 380 dependent)
                    if k % 2 == 1 and len(dq) >= 4:
                        emit_img(dq.popleft())
                        emit_img(dq.popleft())
                    k += 1
            while dq:
                emit_img(dq.popleft())

            out2 = constp.tile([128, ACC_C], f32, tag="out2")
            qtr = ACC_C // 4
            for h in range(4):
                cs = slice(h * qtr, (h + 1) * qtr) if h < 3 else slice(3 * qtr, ACC_C)
                nc.scalar.activation(out2[:, cs], acc_s[:, cs], AF.Sigmoid)
                for t in range(3):
                    nc.sync.dma_start(
                        out=out_d[3 * t:3 * t + 3, cs],
                        in_=out2[32 * t:32 * t + 3, cs],
                    )

            # 1/pmax is folded into the coefficients on the host (exact grid
            # max of a concave quadratic, computed per scanline), so no
            # device-side max and no collective at all.
            if _cache.get("debug"):
                nc.sync.dma_start(out=dbg_img_d[:], in_=acc_s[:])


    nc.compile()
    return nc


def _split3(x):
    """Exact 3-piece bf16 decomposition of an fp32 array."""
    bf = ml_dtypes.bfloat16
    x = x.astype(np.float32)
    p0 = x.astype(bf).astype(np.float32)
    r = x - p0
    p1 = r.astype(bf).astype(np.float32)
    p2 = r - p1
    return p0.astype(bf), p1.astype(bf), p2.astype(bf)


# piece index per pair for coef (i) and basis (j): products with i+j<=2
_PAIR_I = [0, 0, 1, 0, 1, 2]
_PAIR_J = [0, 1, 0, 2, 1, 0]


def _host_prep(mean, alpha, scale, theta, rgb, pixels):
    """Per-gaussian polynomial coefficients + per-pixel basis, bf16-split."""
    mean = np.asarray(mean, np.float64)
    alpha = np.asarray(alpha, np.float64)
    scale = np.asarray(scale, np.float64)
    theta = np.asarray(theta, np.float64)
    rgb = np.asarray(rgb, np.float64)
    pixels = np.asarray(pixels, np.float64)

    ta = 2.0 * np.pi * theta[:, 0]
    c, s = np.cos(ta), np.sin(ta)
    sx2 = scale[:, 0] ** 2
    sy2 = scale[:, 1] ** 2
    a = c * c * sx2 + s * s * sy2
    b = c * s * (sx2 - sy2)
    d = s * s * sx2 + c * c * sy2
    det = a * d - b * b
    A = d / det
    B = a / det
    C = -2.0 * b / det
    lognorm = -np.log(2.0 * np.pi) - 0.5 * np.log(det)
    mux = mean[:, 0, 0] - 0.5
    muy = mean[:, 1, 0] - 0.5

    coef = np.zeros((8, N_GAUSS), np.float64)
    coef[0] = -0.5 * A
    coef[1] = -0.5 * B
    coef[2] = -0.5 * C
    coef[3] = A * mux + 0.5 * C * muy
    coef[4] = B * muy + 0.5 * C * mux
    coef[5] = -0.5 * (A * mux**2 + B * muy**2 + C * mux * muy) + lognorm

    # Exact max of S over the pixel grid, folded into the constant row so
    # the device computes normalized splats directly (no device max, no
    # collective). Exactness: per scanline v=v_h, S is a concave parabola
    # in u, whose grid argmax is the grid point nearest its vertex.
    ug = np.unique(pixels[0, :, 0]) - 0.5 if (
        np.all(pixels[:, :, 0] == pixels[0:1, :, 0])
        and np.all(pixels[:, :, 1] == pixels[:, 0:1, 1])
    ) else None
    if ug is not None and ug.size > 1:
        vg = pixels[:, 0, 1] - 0.5                    # (H,)
        du = ug[1] - ug[0]
        r0 = coef[0][:, None]                         # (N,1)
        bu = coef[2][:, None] * vg[None, :] + coef[3][:, None]   # (N,H)
        cu = (coef[1][:, None] * vg[None, :] ** 2
              + coef[4][:, None] * vg[None, :] + coef[5][:, None])
        ustar = np.clip(-bu / (2.0 * r0), ug[0], ug[-1])
        ustar = ug[0] + np.round((ustar - ug[0]) / du) * du
        smax = float((r0 * ustar**2 + bu * ustar + cu).max())
    else:
        # non-separable pixel grid: brute-force the max on the host
        u = pixels[:, :, 0].reshape(-1) - 0.5
        v = pixels[:, :, 1].reshape(-1) - 0.5
        bas = np.stack([u * u, v * v, u * v, u, v, np.ones_like(u)], 0)
        smax = -np.inf
        for i in range(0, HW, 65536):
            smax = max(smax, float((coef[:6].T @ bas[:, i:i + 65536]).max()))
    coef[5] -= smax
    coef = coef.astype(np.float32)

    u = pixels[:, :, 0].reshape(-1) - 0.5
    v = pixels[:, :, 1].reshape(-1) - 0.5
    basis = np.zeros((8, HW), np.float32)
    basis[0] = u * u
    basis[1] = v * v
    basis[2] = u * v
    basis[3] = u
    basis[4] = v
    basis[5] = 1.0

    cp = _split3(coef)      # 3 x (8, 512) bf16
    bp = _split3(basis)     # 3 x (8, HW) bf16
    coefP = np.zeros((KP, N_GAUSS), ml_dtypes.bfloat16)   # rows 48+ stay 0
    basisP = np.zeros((KP, HW), ml_dtypes.bfloat16)
    for p in range(6):
        coefP[8 * p:8 * p + 8] = cp[_PAIR_I[p]]
        basisP[8 * p:8 * p + 8] = bp[_PAIR_J[p]]

    wfull = (rgb * alpha).astype(np.float32)          # (512, 3)
    w_dev = np.ascontiguousarray(
        wfull.reshape(NCHUNK, 128, 3).transpose(1, 0, 2).reshape(128, 3 * NCHUNK)
    ).astype(ml_dtypes.bfloat16)
    return coefP, basisP, w_dev


def _assemble(results):
    """results[c]["img"] (9, NG*FB) -> full (H, W, 3) image."""
    out = np.empty((HW, 3), np.float32)
    for c in range(NCORES):
        arr = np.asarray(results[c]["img"], np.float32).reshape(3, 3, NG, FB)
        # arr[t, ch, g, u] -> pixel g*F + t*FB + u of this core
        core = arr.transpose(2, 0, 3, 1).reshape(NG * 3 * FB, 3)
        out[c * PX:(c + 1) * PX] = core[:PX]
    return out.reshape(H, W, 3)


def run(inputs, trace=False):
    from concourse.bass_utils import run_bass_kernel_spmd

    coefP, basisP, w_dev = _host_prep(**inputs)
    if "nc" not in _cache:
        _cache["nc"] = _build_nc()
    nc = _cache["nc"]

    in_maps = [
        {
            "basis": np.ascontiguousarray(basisP[:, c * PX:(c + 1) * PX]),
            "coef": coefP,
            "wrgb": w_dev,
        }
        for c in range(NCORES)
    ]
    res = run_bass_kernel_spmd(
        nc, in_maps, core_ids=list(range(NCORES)), trace=trace
    )
    return _assemble(res.results), res


def kernel(mean, alpha, scale, theta, rgb, pixels):
    out, _ = run(
        dict(mean=mean, alpha=alpha, scale=scale, theta=theta, rgb=rgb,
             pixels=pixels)
    )
    return out


# revision 25
# speedup vs baseline: 1.2376x; 1.0198x over previous
"""GaussianImage splat kernel for 8 trn2 NeuronCores (v2).

Math: for gaussian n and pixel p,
  S[n,p] = -0.5 * q[n,p] + ln(norm[n])  (q = mahalanobis quadratic form)
is a degree-2 polynomial in pixel coords: S = coef[6,n]^T @ basis[6,p]
(basis rows u^2, v^2, uv, u, v, 1 with u,v = coords - 0.5).
prob = exp(S); img[p,c] = sum_n prob[n,p]*w[n,c]; out = sigmoid(img/max prob).

fp32 matmul on trn2 PE runs LOW_HIGH dual-pass (~10x slower than bf16), so
the exponent contraction uses an exact 3-piece bf16 split: c = c0+c1+c2,
b = b0+b1+b2 (bf16 pieces), keeping the 6 products with i+j<=2 gives
~2^-27 relative error — better than an fp32 matmul. The 6 (ci,bj) pairs
are stacked on the contraction axis: one K=48 bf16 matmul.

Sharding: pixels split across 8 cores (each core handles all 512 gaussians
on its 32768 pixels): no image all-reduce; only a 512B AllReduce(max).

Per core, per (pixel-group g of 1024 px, gaussian-chunk j of 128):
  PE:  S_psum[128, 1024] = coefP_j[48,128]^T @ basisP[48, 1024]  (bf16)
  ACT: prob[128, 1024] = exp(S_psum) -> SBUF bf16
  DVE: running max: run = max(run, prob)   (tensor_tensor, 2x bf16)
  PE:  img[3(q), 512] += w_j[128,3]^T @ prob[:, t*512:...]  (bf16)
       with q = g%4 selecting PSUM col-group 32q via tile_position, so 4
       groups accumulate into one [128, 1024] psum tile (2 banks).
Per super (4 groups): one DVE copy [128,1024] psum->sbuf acc.
Tail: reduce run -> [128,1], AllReduce(max), reciprocal, broadcast,
  sigmoid(acc * 1/pmax) in one [128, 8192] ACT op, DMA 4x[3,8192] out.
"""

import sys

if "/opt/trn_rl_repo" not in sys.path:
    sys.path.insert(0, "/opt/trn_rl_repo")

import numpy as np
import ml_dtypes

N_GAUSS = 512
H = W = 512
HW = H * W
NCORES = 8
PX = HW // NCORES          # 32768 pixels per core
F = 1536                   # pixels per exp group (3 psum banks); the 3
                           # 512-px blocks of a group land in PE col-groups
                           # 0/32/64 of one [128,512] img psum tile (col
                           # group 96 is a broken HW quadrant)
FB = 512                   # pixels per block / img matmul
NG = (PX + F - 1) // F     # 22 groups (21x1536 + 1x512)
NCHUNK = 4                 # gaussian chunks of 128
KP = 128                   # 6 bf16 piece-pairs x 8 rows, zero-padded to 128:
                           # partial-row-group matmuls (K=48) measure ~486ns
                           # vs ~283ns for full-array K=128 on HW
ACC_C = NG * FB            # acc columns (one [128,512] img tile per group)

_cache = {}


def _build_nc():
    import concourse.mybir as mybir
    from concourse import bacc, tile

    f32 = mybir.dt.float32
    bf16 = mybir.dt.bfloat16
    AF = mybir.ActivationFunctionType
    ALU = mybir.AluOpType

    nc = bacc.Bacc("TRN2", num_devices=NCORES)

    basis_d = nc.dram_tensor("basis", [KP, PX], bf16, kind="ExternalInput")
    coef_d = nc.dram_tensor("coef", [KP, 512], bf16, kind="ExternalInput")
    w_d = nc.dram_tensor("wrgb", [128, 3 * NCHUNK], bf16, kind="ExternalInput")
    out_d = nc.dram_tensor("img", [9, ACC_C], f32, kind="ExternalOutput")
    if _cache.get("debug"):
        dbg_img_d = nc.dram_tensor("dbg_img", [128, ACC_C], f32, kind="ExternalOutput")

    with tile.TileContext(nc) as tc:
        with (
            tc.tile_pool(name="const", bufs=1) as constp,
            tc.tile_pool(name="probp", bufs=8) as probp,
            tc.tile_pool(name="psS", bufs=2, space="PSUM") as psS,
            tc.tile_pool(name="psI", bufs=2, space="PSUM") as psI,
        ):
            basis_s = constp.tile([KP, PX], bf16, tag="basis")
            coef_s = constp.tile([KP, 512], bf16, tag="coef")
            w_s = constp.tile([128, 3 * NCHUNK], bf16, tag="w")
            acc_s = constp.tile([128, ACC_C], f32, tag="acc")
            zrow = constp.tile([1, 128], bf16, tag="zrow")

            # stage inputs: weights first (first matmul needs them), then
            # basis in graduated chunks so group 0 lands fast while later
            # groups amortize the ~0.7us per-DMA issue cost on the SP queue.
            # coef split by gaussian chunk: the first matmul only needs
            # chunk 0's columns, so it can start ~4us earlier
            for j4 in range(4):
                cs = slice(128 * j4, 128 * j4 + 128)
                nc.sync.dma_start(out=coef_s[:, cs], in_=coef_d[:, cs])
            nc.sync.dma_start(out=w_s[:], in_=w_d[:])
            for p4 in range(4):
                ps = slice(32 * p4, 32 * p4 + 32)
                nc.sync.dma_start(out=basis_s[ps, 0:512], in_=basis_d[ps, 0:512])
            edges = [512]
            for sz in [512, 1024, 1024, 1024, 2048, 2048]:
                edges.append(edges[-1] + sz)
            while edges[-1] < PX:
                edges.append(min(PX, edges[-1] + 4096))
            for e0, e1 in zip(edges[:-1], edges[1:]):
                nc.sync.dma_start(
                    out=basis_s[:, e0:e1], in_=basis_d[:, e0:e1]
                )
            nc.vector.memset(zrow[:], 0.0)

            def emit_img(p):
                # deferred a few (g,j) iterations: prob is long since ready,
                # so PE never stalls waiting on ACT (software pipeline).
                pimg, pj, pprob, pf, pflush, pg = p
                for t in range(pf // FB):
                    nc.tensor.matmul(
                        pimg[32 * t:32 * t + 3, :],
                        w_s[:, 3 * pj:3 * pj + 3],
                        pprob[:, t * FB:(t + 1) * FB],
                        start=False,
                        stop=pflush and not _cache.get("debug"),
                        skip_group_check=True,
                    )
                if pflush:
                    nc.vector.tensor_copy(
                        acc_s[:, pg * FB:(pg + 1) * FB], pimg[:]
                    )

            from collections import deque

            dq = deque()
            k = 0
            for g in range(NG):
                p0 = g * F
                f = min(F, PX - p0)
                img = psI.tile([128, FB], f32, tag="img")
                # zeroing matmul: writes 0 to every element of the bank and
                # sets has_written on all 128 partitions, so the col-group
                # accumulation below is pure accumulate regardless of the
                # HW's bank-clear partition scope.
                nc.tensor.matmul(
                    img[:, :],
                    zrow[:],
                    basis_s[0:1, 0:FB],
                    start=True,
                    stop=False,
                    skip_group_check=True,
                )
                for j in range(NCHUNK):
                    S = psS.tile([128, f], f32, tag="S")
                    prob = probp.tile([128, f], bf16, tag="prob")
                    for t in range(f // 512):
                        nc.tensor.matmul(
                            S[:, t * 512:(t + 1) * 512],
                            coef_s[:, j * 128:(j + 1) * 128],
                            basis_s[:, p0 + t * 512: p0 + (t + 1) * 512],
                            start=True,
                            stop=True,
                        )
                    nc.scalar.activation(prob[:], S[:], AF.Exp)
                    dq.append((img, j, prob, f, j == NCHUNK - 1, g))
                    # emit deferred img matmuls two iterations at a time so
                    # PE sees batched same-shape runs (measured 222ns/MM
                    # batched vs 282 alternating vs 380 dependent)
                    if k % 3 == 2 and len(dq) >= 6:
                        emit_img(dq.popleft())
                        emit_img(dq.popleft())
                        emit_img(dq.popleft())
                    k += 1
            while dq:
                emit_img(dq.popleft())

            out2 = constp.tile([128, ACC_C], f32, tag="out2")
            qtr = ACC_C // 4
            for h in range(4):
                cs = slice(h * qtr, (h + 1) * qtr) if h < 3 else slice(3 * qtr, ACC_C)
                nc.scalar.activation(out2[:, cs], acc_s[:, cs], AF.Sigmoid)
                for t in range(3):
                    nc.sync.dma_start(
                        out=out_d[3 * t:3 * t + 3, cs],
                        in_=out2[32 * t:32 * t + 3, cs],
                    )

            # 1/pmax is folded into the coefficients on the host (exact grid
            # max of a concave quadratic, computed per scanline), so no
            # device-side max and no collective at all.
            if _cache.get("debug"):
                nc.sync.dma_start(out=dbg_img_d[:], in_=acc_s[:])


    nc.compile()
    return nc


def _split3(x):
    """Exact 3-piece bf16 decomposition of an fp32 array."""
    bf = ml_dtypes.bfloat16
    x = x.astype(np.float32)
    p0 = x.astype(bf).astype(np.float32)
    r = x - p0
    p1 = r.astype(bf).astype(np.float32)
    p2 = r - p1
    return p0.astype(bf), p1.astype(bf), p2.astype(bf)


# piece index per pair for coef (i) and basis (j): products with i+j<=2
_PAIR_I = [0, 0, 1, 0, 1, 2]
_PAIR_J = [0, 1, 0, 2, 1, 0]


def _host_prep(mean, alpha, scale, theta, rgb, pixels):
    """Per-gaussian polynomial coefficients + per-pixel basis, bf16-split."""
    mean = np.asarray(mean, np.float64)
    alpha = np.asarray(alpha, np.float64)
    scale = np.asarray(scale, np.float64)
    theta = np.asarray(theta, np.float64)
    rgb = np.asarray(rgb, np.float64)
    pixels = np.asarray(pixels, np.float64)

    ta = 2.0 * np.pi * theta[:, 0]
    c, s = np.cos(ta), np.sin(ta)
    sx2 = scale[:, 0] ** 2
    sy2 = scale[:, 1] ** 2
    a = c * c * sx2 + s * s * sy2
    b = c * s * (sx2 - sy2)
    d = s * s * sx2 + c * c * sy2
    det = a * d - b * b
    A = d / det
    B = a / det
    C = -2.0 * b / det
    lognorm = -np.log(2.0 * np.pi) - 0.5 * np.log(det)
    mux = mean[:, 0, 0] - 0.5
    muy = mean[:, 1, 0] - 0.5

    coef = np.zeros((8, N_GAUSS), np.float64)
    coef[0] = -0.5 * A
    coef[1] = -0.5 * B
    coef[2] = -0.5 * C
    coef[3] = A * mux + 0.5 * C * muy
    coef[4] = B * muy + 0.5 * C * mux
    coef[5] = -0.5 * (A * mux**2 + B * muy**2 + C * mux * muy) + lognorm

    # Exact max of S over the pixel grid, folded into the constant row so
    # the device computes normalized splats directly (no device max, no
    # collective). Exactness: per scanline v=v_h, S is a concave parabola
    # in u, whose grid argmax is the grid point nearest its vertex.
    ug = np.unique(pixels[0, :, 0]) - 0.5 if (
        np.all(pixels[:, :, 0] == pixels[0:1, :, 0])
        and np.all(pixels[:, :, 1] == pixels[:, 0:1, 1])
    ) else None
    if ug is not None and ug.size > 1:
        vg = pixels[:, 0, 1] - 0.5                    # (H,)
        du = ug[1] - ug[0]
        r0 = coef[0][:, None]                         # (N,1)
        bu = coef[2][:, None] * vg[None, :] + coef[3][:, None]   # (N,H)
        cu = (coef[1][:, None] * vg[None, :] ** 2
              + coef[4][:, None] * vg[None, :] + coef[5][:, None])
        ustar = np.clip(-bu / (2.0 * r0), ug[0], ug[-1])
        ustar = ug[0] + np.round((ustar - ug[0]) / du) * du
        smax = float((r0 * ustar**2 + bu * ustar + cu).max())
    else:
        # non-separable pixel grid: brute-force the max on the host
        u = pixels[:, :, 0].reshape(-1) - 0.5
        v = pixels[:, :, 1].reshape(-1) - 0.5
        bas = np.stack([u * u, v * v, u * v, u, v, np.ones_like(u)], 0)
        smax = -np.inf
        for i in range(0, HW, 65536):
            smax = max(smax, float((coef[:6].T @ bas[:, i:i + 65536]).max()))
    coef[5] -= smax
    coef = coef.astype(np.float32)

    u = pixels[:, :, 0].reshape(-1) - 0.5
    v = pixels[:, :, 1].reshape(-1) - 0.5
    basis = np.zeros((8, HW), np.float32)
    basis[0] = u * u
    basis[1] = v * v
    basis[2] = u * v
    basis[3] = u
    basis[4] = v
    basis[5] = 1.0

    cp = _split3(coef)      # 3 x (8, 512) bf16
    bp = _split3(basis)     # 3 x (8, HW) bf16
    coefP = np.zeros((KP, N_GAUSS), ml_dtypes.bfloat16)   # rows 48+ stay 0
    basisP = np.zeros((KP, HW), ml_dtypes.bfloat16)
    for p in range(6):
        coefP[8 * p:8 * p + 8] = cp[_PAIR_I[p]]
        basisP[8 * p:8 * p + 8] = bp[_PAIR_J[p]]

    wfull = (rgb * alpha).astype(np.float32)          # (512, 3)
    w_dev = np.ascontiguousarray(
        wfull.reshape(NCHUNK, 128, 3).transpose(1, 0, 2).reshape(128, 3 * NCHUNK)
    ).astype(ml_dtypes.bfloat16)
    return coefP, basisP, w_dev


def _assemble(results):
    """results[c]["img"] (9, NG*FB) -> full (H, W, 3) image."""
    out = np.empty((HW, 3), np.float32)
    for c in range(NCORES):
        arr = np.asarray(results[c]["img"], np.float32).reshape(3, 3, NG, FB)
        # arr[t, ch, g, u] -> pixel g*F + t*FB + u of this core
        core = arr.transpose(2, 0, 3, 1).reshape(NG * 3 * FB, 3)
        out[c * PX:(c + 1) * PX] = core[:PX]
    return out.reshape(H, W, 3)


def run(inputs, trace=False):
    from concourse.bass_utils import run_bass_kernel_spmd

    coefP, basisP, w_dev = _host_prep(**inputs)
    if "nc" not in _cache:
        _cache["nc"] = _build_nc()
    nc = _cache["nc"]

    in_maps = [
        {
            "basis": np.ascontiguousarray(basisP[:, c * PX:(c + 1) * PX]),
            "coef": coefP,
            "wrgb": w_dev,
        }
        for c in range(NCORES)
    ]
    res = run_bass_kernel_spmd(
        nc, in_maps, core_ids=list(range(NCORES)), trace=trace
    )
    return _assemble(res.results), res


def kernel(mean, alpha, scale, theta, rgb, pixels):
    out, _ = run(
        dict(mean=mean, alpha=alpha, scale=scale, theta=theta, rgb=rgb,
             pixels=pixels)
    )
    return out
